# revision 31
# baseline (speedup 1.0000x reference)
"""Causal self-attention kernel for Trainium2, sharded over 8 NeuronCores.

Problem: B=4, T=2048, DIM=1024, H=16 heads, head_dim=64, fp32 I/O.

Sharding: (batch, head-group) pairs -> 8 shards. Core c handles batch
b = c//2 and head group g = c%2 (8 heads each). Each core computes its
q/k/v projections for its head slice, causal flash-style attention, and
a partial o_proj against its head-slice of wo. The host sums the two
partial o_proj outputs per batch (the "all-reduce") while gathering.

Pipeline strategy (per core): T is processed in 4 chunks of 512. Chunk
c's attention is interleaved with chunk c+1's q/k/v projections and
chunk c-1's o_proj so the tensor engine never idles long enough for the
HAM clock gate to re-throttle it to 1.2 GHz.

PE scheduling (the main perf lever): the PE pipelines back-to-back
matmuls of the same row-configuration at the streaming roofline
(N cycles each), but every transition between configurations (score
K=64 <-> AV K=128 <-> projection K=128-different-operands) costs a
~105-160ns fill/drain bubble. So attention processes j (key-tile)
indices in PAIRS: 4 score matmuls back-to-back (the two heads of a
pair overlap via row-tiling: head A in array rows 0-63, head B in
64-127), then the 4 AV matmuls of the previous j-pair, then a batch of
projection fillers -- 3 transitions per 2 j's instead of 3 per j.

Per-core layout:
  - Host pre-transposes x and the weight slices so the contraction dim
    lands on SBUF partitions, and casts to bf16.
  - Scores are computed TRANSPOSED: sT[tk, tq] = k @ q^T, so softmax'd
    probabilities come out with tk on partitions -- the layout the
    attn@v matmul needs as its moving operand (lhsT = v).
  - The two heads of a pair occupy partitions 0-63 / 64-127 of the same
    QT/KT tile; their scores land in one [128, 1024] psum tile (head A
    cols 0-511, head B cols 512-1023) so ONE scalar-engine exp covers
    both heads (halves ACT instruction count).
  - Softmax skips max-subtraction (scores are O(1) by construction),
    the denominator comes free from a ones column appended to v, and
    1/denom uses the fast DVE reciprocal instead of ACT Ln/Exp.
  - Causal masking inside diagonal 128-tiles: DVE multiply with a
    0/1 lower-triangle mask after the exp.
  - y is produced in bf16 (halves output DMA; adds ~0.2% rounding,
    well within the 2e-2 budget); host upcasts while summing the
    tensor-parallel partials.
"""

import numpy as np
import ml_dtypes

import concourse.bass as bass
import concourse.bacc as bacc
import concourse.mybir as mybir
import concourse.tile as tile
from concourse.bass import ds, ts
from concourse.bass_utils import run_bass_kernel_spmd

BF16 = mybir.dt.bfloat16
F32 = mybir.dt.float32

T = 2048
D = 1024
DG = 512          # head-group width (8 heads x 64)
NH = 8            # heads per core
DH = 64
P = 128
NKO = D // P      # 8 contraction tiles for projections
W = 512           # tq chunk width
NCH = T // W      # 4 chunks
NTC = W // P      # 4 t-tiles per chunk
NPAIR = NH // 2   # 4 head pairs

_CACHED = None  # (nc, input names) -- build/trace once per process


def _build_kernel():
    nc = bacc.Bacc("TRN2", target_bir_lowering=False, debug=False)

    # inputs come host-tiled so every DMA reads contiguous HBM
    xT_d = nc.dram_tensor("xT", [NKO, NCH, P, W], BF16, kind="ExternalInput").ap()
    wqT_d = nc.dram_tensor("wqT", [NKO, P, DG], BF16, kind="ExternalInput").ap()
    wkT_d = nc.dram_tensor("wkT", [NKO, P, DG], BF16, kind="ExternalInput").ap()
    wvT_d = nc.dram_tensor("wvT", [NKO, P, DG], BF16, kind="ExternalInput").ap()
    woT_d = nc.dram_tensor("woT", [DG // P, P, D], BF16, kind="ExternalInput").ap()
    y_d = nc.dram_tensor("y", [T, D], BF16, kind="ExternalOutput").ap()

    with tile.TileContext(nc) as tc:
        with (
            tc.tile_pool(name="const", bufs=1) as const,
            tc.tile_pool(name="sb", bufs=1) as sb,
            tc.tile_pool(name="work", bufs=6) as work,
            tc.tile_pool(name="wnorm", bufs=4) as wnorm,
            tc.tile_pool(name="ysbp", bufs=8) as ysbp,
            tc.tile_pool(name="ps", bufs=2, space="PSUM") as psp,
            tc.tile_pool(name="av", bufs=2, space="PSUM") as avp,
            tc.tile_pool(name="pj", bufs=2, space="PSUM") as pjp,
        ):
            # ---- constants ----
            dmy = const.tile([P, W], BF16, tag="dmy")
            nc.vector.memset(dmy, 0.0)
            mskb = const.tile([P, P], BF16, tag="mskb")

            # ---- persistent SBUF tensors ----
            XT = sb.tile([P, NKO, T], BF16, tag="XT")
            WQT = sb.tile([P, NKO, DG], BF16, tag="WQT")
            WKT = sb.tile([P, NKO, DG], BF16, tag="WKT")
            WVT = sb.tile([P, NKO, DG], BF16, tag="WVT")
            WOT = sb.tile([P, DG // P, D], BF16, tag="WOT")
            QT = sb.tile([P, DG // P, T], BF16, tag="QT")
            KT = sb.tile([P, DG // P, T], BF16, tag="KT")
            VA = sb.tile([P, T // P, NH, DH + 1], BF16, tag="VA")
            # one tile per head pair: o_proj's jt-accumulation then only
            # waits on the pair it actually reads
            OGT = [
                sb.tile([P, T], BF16, name=f"OGT{p}", tag=f"OGT{p}")
                for p in range(NPAIR)
            ]

            # warm the PE's HAM clock gate while the first DMAs land:
            # ~12 dummy matmuls keep it busy past the 3.4us activity
            # window so real work starts at 2.4 GHz, not 1.2
            wps = psp.tile([P, 2 * W], F32, tag="s")
            for _ in range(12):
                nc.tensor.matmul(
                    wps[:, 0:W], lhsT=dmy[:, 0:P], rhs=dmy,
                    start=True, stop=True,
                )

            # ---- input DMAs ----
            # a single sequencer takes ~600ns per dma_start issue, so a
            # one-engine stream of 24+ transfers trickles out over 14us
            # and starves the PE during the ramp (each resulting gap also
            # re-throttles the HAM clock, which then needs ~3.4us at half
            # speed to re-warm). Issue the four early input streams from
            # FOUR sequencers in parallel so everything chunk-0 needs is
            # in flight within ~4us.
            for k in range(NKO):
                nc.sync.dma_start(XT[:, k, 0:W], xT_d[k, 0])
                nc.scalar.dma_start(WQT[:, k, :], wqT_d[k])
                nc.gpsimd.dma_start(WKT[:, k, :], wkT_d[k])
            for k in range(NKO):
                eng = nc.sync if k % 2 == 0 else nc.scalar
                eng.dma_start(WVT[:, k, :], wvT_d[k])
            for k in range(NKO):
                nc.sync.dma_start(XT[:, k, ds(W, W)], xT_d[k, 1])
            for j in range(DG // P):
                nc.scalar.dma_start(WOT[:, j, :], woT_d[j])
            for c in range(2, NCH):
                for k in range(NKO):
                    nc.sync.dma_start(XT[:, k, ds(c * W, W)], xT_d[k, c])

            # gpsimd constants AFTER its DMA issues so wk goes out first;
            # neither is needed until attention starts.
            # multiplicative causal mask for diag tiles: 1 where tq >= tk
            nc.gpsimd.memset(mskb, 1.0)
            nc.gpsimd.affine_select(
                out=mskb, in_=mskb,
                compare_op=mybir.AluOpType.is_ge,
                fill=0.0, base=0,
                pattern=[[1, P]], channel_multiplier=-1,
            )
            # v_aug ones column
            nc.gpsimd.memset(VA[:, :, :, DH], 1.0)

            # ---- projection / o_proj emitters (also used as PE filler) ----
            def proj_qk(wsb, dst, c, dg, warm=False):
                ps = pjp.tile([P, W], F32, tag="pj")
                for k in range(NKO):
                    nc.tensor.matmul(
                        ps,
                        lhsT=wsb[:, k, ts(dg, P)],
                        rhs=XT[:, k, ds(c * W, W)],
                        start=(k == 0), stop=(k == NKO - 1),
                    )
                    if warm:
                        # chunk-0 ramp: the per-k DMA tiles land ~0.7us
                        # apart, slower than the PE consumes them; dummy
                        # matmuls between the real ones keep the PE busy
                        # so the HAM clock gate never re-throttles
                        for _ in range(2):
                            nc.tensor.matmul(
                                wps[:, 0:P], lhsT=dmy[:, 0:P],
                                rhs=dmy[:, 0:P], start=True, stop=True,
                            )
                nc.vector.tensor_copy(dst[:, dg, ds(c * W, W)], ps)

            def proj_v(c, tl):
                tt = c * NTC + tl
                ps = pjp.tile([P, W], F32, tag="pj")
                for k in range(NKO):
                    nc.tensor.matmul(
                        ps,
                        lhsT=XT[:, k, ts(tt, P)],
                        rhs=WVT[:, k, :],
                        start=(k == 0), stop=(k == NKO - 1),
                    )
                nc.vector.tensor_copy(
                    VA[:, tt, :, 0:DH],
                    ps.rearrange("p (h d) -> p h d", h=NH),
                )

            def proj_groups(c):
                gs = []
                for dg in range(DG // P):
                    gs.append(lambda dg=dg: proj_qk(WQT, QT, c, dg))
                for dg in range(DG // P):
                    gs.append(lambda dg=dg: proj_qk(WKT, KT, c, dg))
                for tl in range(NTC):
                    gs.append(lambda tl=tl: proj_v(c, tl))
                return gs

            def oproj_tt(c, tl):
                tt = c * NTC + tl
                last = c == NCH - 1
                ysb = ysbp.tile([P, D], BF16, tag="ysb")
                for piece in range(2):
                    ps = pjp.tile([P, W], F32, tag="pj")
                    for jt in range(DG // P - 1):
                        nc.tensor.matmul(
                            ps,
                            lhsT=OGT[jt][:, ts(tt, P)],
                            rhs=WOT[:, jt, ds(piece * W, W)],
                            start=(jt == 0), stop=False,
                        )
                    jt = DG // P - 1
                    nc.tensor.matmul(
                        ps,
                        lhsT=OGT[jt][:, ts(tt, P)],
                        rhs=WOT[:, jt, ds(piece * W, W)],
                        start=False, stop=True,
                    )
                    nc.vector.tensor_copy(ysb[:, ds(piece * W, W)], ps)
                    # per-piece issue, split across DMA queues: transfer
                    # time gates both ysb reuse and the kernel's tail.
                    # for the last chunk, alternate issuing engines so the
                    # final drain is not serialized on one sequencer
                    nsplit = 4 if last else 2
                    wq_ = W // nsplit
                    for q in range(nsplit):
                        o = piece * W + q * wq_
                        eng = nc.scalar if last and q % 2 else nc.sync
                        eng.dma_start(
                            y_d[ts(tt, P), ds(o, wq_)], ysb[:, ds(o, wq_)])

            def oproj_groups(c):
                return [lambda tl=tl: oproj_tt(c, tl) for tl in range(NTC)]

            # ---- attention ----
            LAG = 2  # j2-iterations of score/exp lookahead before each AV

            def emit_av_quad(pair, avA, avB, quad, jmax):
                # both heads, both j's of the pair; per-head adjacency
                # keeps same-psum-bank matmuls back-to-back
                for h, av in ((0, avA), (1, avB)):
                    for j, et, off, boff, w in quad:
                        nc.tensor.matmul(
                            av[0:DH + 1, ds(off, w)],
                            lhsT=VA[:, j, 2 * pair + h, :],
                            rhs=et[:, ds(h * boff, w)],
                            start=(j == 0),
                            stop=(j == jmax),
                        )

            # The normalize chain (den -> 1/den -> broadcast -> multiply)
            # is latency-heavy and the DVE queue is strict FIFO: issuing
            # it inline at a pair boundary parks the reciprocal/multiply
            # at the DVE queue head waiting on the pair's last AV matmul,
            # blocking the projection psum-evacuation copies behind it
            # (which stalls the PE on psum banks). Instead the chain is
            # staged: den copies go to ACT at the pair boundary (zero
            # blocking there), and the DVE/GPSIMD stages are deferred via
            # post_q -- one stage per j2-iteration of the NEXT pair -- so
            # every stage's input is already complete when it dequeues.
            post_q = []

            def normalize_start(avA, avB, dstA, dstB):
                state = {}

                def s0():
                    # by now the pair's last AV long completed: these
                    # dequeue with their semaphores already fired
                    denA = wnorm.tile([1, W], F32, tag="den")
                    denB = wnorm.tile([1, W], F32, tag="den")
                    nc.vector.tensor_copy(denA, avA[DH:DH + 1, :])
                    nc.vector.tensor_copy(denB, avB[DH:DH + 1, :])
                    state["den"] = (denA, denB)

                def s1():
                    denA, denB = state["den"]
                    recA = wnorm.tile([1, W], F32, tag="rec")
                    recB = wnorm.tile([1, W], F32, tag="rec")
                    nc.vector.reciprocal_approx_fast(recA, denA)
                    nc.vector.reciprocal_approx_fast(recB, denB)
                    bcbA = wnorm.tile([DH, W], F32, tag="bcb")
                    bcbB = wnorm.tile([DH, W], F32, tag="bcb")
                    nc.gpsimd.partition_broadcast(bcbA, recA)
                    nc.gpsimd.partition_broadcast(bcbB, recB)
                    state["bcb"] = (bcbA, bcbB)

                def s2():
                    bcbA, bcbB = state["bcb"]
                    # multiply straight from psum: the av banks free here
                    nc.vector.tensor_mul(dstA, avA[0:DH, :], bcbA)
                    nc.vector.tensor_mul(dstB, avB[0:DH, :], bcbB)

                post_q.extend([s0, s1, s2])

            def pump_post():
                if post_q:
                    post_q.pop(0)()

            def attention_pair(pair, c, pull_filler):
                jmax = (c + 1) * NTC - 1
                avA = avp.tile([P, W], F32, tag="av")
                avB = avp.tile([P, W], F32, tag="av")
                pend = []
                for j2 in range((jmax + 1) // 2):
                    quad = []
                    for j in (2 * j2, 2 * j2 + 1):
                        off = max(0, j * P - c * W)
                        w = W - off
                        lo = max(c * W, j * P)
                        diag = j * P >= c * W
                        boff = W
                        ps = psp.tile([P, 2 * W], F32, tag="s")
                        for h in range(2):
                            nc.tensor.matmul(
                                ps[:, ds(h * boff, w)],
                                lhsT=KT[h * DH:(h + 1) * DH, pair, ts(j, P)],
                                rhs=QT[h * DH:(h + 1) * DH, pair, ds(lo, w)],
                                start=True, stop=True,
                            )
                        quad.append((j, ps, off, boff, w, diag))
                    done = []
                    for j, ps, off, boff, w, diag in quad:
                        et = work.tile([P, 2 * W], BF16, tag="et")
                        if w <= 256:
                            # narrow diag tiles: two gap-free exps beat
                            # one spanning the dead [w, W) columns
                            for h in range(2):
                                nc.scalar.activation(
                                    et[:, ds(h * W, w)], ps[:, ds(h * W, w)],
                                    mybir.ActivationFunctionType.Exp,
                                    scale=0.125,
                                )
                        else:
                            nc.scalar.activation(
                                et[:, 0:W + w], ps[:, 0:W + w],
                                mybir.ActivationFunctionType.Exp,
                                scale=0.125,
                            )
                        if diag:
                            nc.vector.tensor_mul(et[:, 0:P], et[:, 0:P], mskb)
                            nc.vector.tensor_mul(
                                et[:, ds(boff, P)], et[:, ds(boff, P)], mskb)
                        done.append((j, et, off, boff, w))
                    pend.append(done)
                    # pump BEFORE the AV quad: the av-bank-freeing muls
                    # of the previous pair land just ahead of the AV
                    # writes that reuse those banks
                    pump_post()
                    if len(pend) > LAG:
                        # emit TWO quads (8 AVs back-to-back): AVs all
                        # accumulate into the same two psum banks, so
                        # batching costs nothing and halves the number
                        # of row-configuration transitions the PE pays.
                        # fillers then ride on the OPPOSITE iteration
                        # parity, so a typical iteration has only two
                        # config transitions instead of three.
                        emit_av_quad(pair, avA, avB, pend.pop(0), jmax)
                        emit_av_quad(pair, avA, avB, pend.pop(0), jmax)
                    pull_filler()
                # chunk-0 pairs have only 2 iterations -- finish the
                # previous pair's chain before this pair's AV drain
                while post_q:
                    post_q.pop(0)()
                for quad in pend:
                    emit_av_quad(pair, avA, avB, quad, jmax)
                # engines support a shifted output partition base: head B's
                # normalized output goes straight into partitions 64-127
                normalize_start(
                    avA, avB,
                    OGT[pair][0:DH, ds(c * W, W)],
                    OGT[pair][DH:P, ds(c * W, W)],
                )
                if pair == NPAIR - 1 and c == NCH - 1:
                    # final pair gates the last o_proj: run the chain now
                    while post_q:
                        post_q.pop(0)()

            # ---- main schedule ----
            # only q+k of chunk 0 up front: scores don't need v, and the
            # v-projection otherwise serializes attention start behind the
            # late-arriving wv DMAs. q/k alternate per dg (pair p consumes
            # dg=p), and the first groups carry warm-keeping dummies
            # through the DMA-trickle ramp.
            for dg in range(DG // P):
                proj_qk(WQT, QT, 0, dg, warm=(dg < 2))
                proj_qk(WKT, KT, 0, dg, warm=(dg < 2))

            # Filler assignment balances PE work against the scalar
            # engine's exp load, which grows with the causal window:
            # chunk 3 is exp-bound (~50us of ACT vs ~22us of attention
            # matmuls), so ALL deferred o_projs run there; chunks 0-2
            # take the q/k/v projections.
            for c in range(NCH):
                fillers = []
                if c == 0:
                    # chunk-0 v-projection rides as the first filler; the
                    # AVs that need it start an iteration in
                    fillers += proj_groups(0)[8:]
                if c + 1 < NCH:
                    fillers += proj_groups(c + 1)
                if c == NCH - 1:
                    for cc in range(NCH - 1):
                        fillers += oproj_groups(cc)
                total_slots = NPAIR * ((c + 1) * NTC // 2)
                state = {"slot": 0, "done": 0}

                def pull_filler(pull=True):
                    state["slot"] += 1
                    if not pull:
                        return
                    want = min(
                        len(fillers),
                        len(fillers) * state["slot"] // total_slots,
                    )
                    while state["done"] < want:
                        fillers[state["done"]]()
                        state["done"] += 1

                for pair in range(NPAIR):
                    attention_pair(pair, c, pull_filler)
                while state["done"] < len(fillers):
                    fillers[state["done"]]()
                    state["done"] += 1

            for g in oproj_groups(NCH - 1):
                g()

    nc.compile()
    return nc


def _get_nc():
    global _CACHED
    if _CACHED is None:
        _CACHED = _build_kernel()
    return _CACHED


def _shard_inputs(x, wq, wk, wv, wo):
    bf = ml_dtypes.bfloat16

    def tile_w(wT):  # [D, DG] -> [NKO, P, DG]
        return np.ascontiguousarray(wT.reshape(NKO, P, DG)).astype(bf)

    in_maps = []
    for core in range(8):
        b, g = divmod(core, 2)
        gs = slice(g * DG, (g + 1) * DG)
        xT = x[b].T  # [D, T]
        x4 = xT.reshape(NKO, P, NCH, W).transpose(0, 2, 1, 3)
        in_maps.append({
            "xT": np.ascontiguousarray(x4).astype(bf),
            "wqT": tile_w(wq[gs, :].T),
            "wkT": tile_w(wk[gs, :].T),
            "wvT": tile_w(wv[gs, :].T),
            "woT": np.ascontiguousarray(
                wo[:, gs].T.reshape(DG // P, P, D)).astype(bf),
        })
    return in_maps


def kernel(x, wq, wk, wv, wo, _trace=False, _trace_cores=None):
    x = np.asarray(x, dtype=np.float32)
    wq = np.asarray(wq, dtype=np.float32)
    wk = np.asarray(wk, dtype=np.float32)
    wv = np.asarray(wv, dtype=np.float32)
    wo = np.asarray(wo, dtype=np.float32)

    nc = _get_nc()
    in_maps = _shard_inputs(x, wq, wk, wv, wo)
    res = run_bass_kernel_spmd(
        nc, in_maps, core_ids=list(range(8)),
        trace=_trace,
        **({"trace_cores": _trace_cores} if _trace_cores else {}),
    )
    B = x.shape[0]
    y = np.zeros((B, T, D), dtype=np.float32)
    for core in range(8):
        b = core // 2
        y[b] += res.results[core]["y"].astype(np.float32)
    if _trace:
        return y, res
    return y


# revision 33
# speedup vs baseline: 1.1955x; 1.1955x over previous
"""Causal self-attention kernel for Trainium2, sharded over 8 NeuronCores.

Problem: B=4, T=2048, DIM=1024, H=16 heads, head_dim=64, fp32 I/O.

Sharding: (batch, head-group) pairs -> 8 shards. Core c handles batch
b = c//2 and head group g = c%2 (8 heads each). Each core computes its
q/k/v projections for its head slice, causal flash-style attention, and
a partial o_proj against its head-slice of wo. The host sums the two
partial o_proj outputs per batch (the "all-reduce") while gathering.

Pipeline strategy (per core): T is processed in 4 chunks of 512. Chunk
c's attention is interleaved with chunk c+1's q/k/v projections and
chunk c-1's o_proj so the tensor engine never idles long enough for the
HAM clock gate to re-throttle it to 1.2 GHz.

PE scheduling (the main perf lever): the PE pipelines back-to-back
matmuls of the same row-configuration at the streaming roofline
(N cycles each), but every transition between configurations (score
K=64 <-> AV K=128 <-> projection K=128-different-operands) costs a
~105-160ns fill/drain bubble. So attention processes j (key-tile)
indices in PAIRS: 4 score matmuls back-to-back (the two heads of a
pair overlap via row-tiling: head A in array rows 0-63, head B in
64-127), then the 4 AV matmuls of the previous j-pair, then a batch of
projection fillers -- 3 transitions per 2 j's instead of 3 per j.

Per-core layout:
  - Host pre-transposes x and the weight slices so the contraction dim
    lands on SBUF partitions, and casts to bf16.
  - Scores are computed TRANSPOSED: sT[tk, tq] = k @ q^T, so softmax'd
    probabilities come out with tk on partitions -- the layout the
    attn@v matmul needs as its moving operand (lhsT = v).
  - The two heads of a pair occupy partitions 0-63 / 64-127 of the same
    QT/KT tile; their scores land in one [128, 1024] psum tile (head A
    cols 0-511, head B cols 512-1023) so ONE scalar-engine exp covers
    both heads (halves ACT instruction count).
  - Softmax skips max-subtraction (scores are O(1) by construction),
    the denominator comes free from a ones column appended to v, and
    1/denom uses the fast DVE reciprocal instead of ACT Ln/Exp.
  - Causal masking inside diagonal 128-tiles: DVE multiply with a
    0/1 lower-triangle mask after the exp.
  - y is produced in bf16 (halves output DMA; adds ~0.2% rounding,
    well within the 2e-2 budget); host upcasts while summing the
    tensor-parallel partials.
"""

import numpy as np
import ml_dtypes

import concourse.bass as bass
import concourse.bacc as bacc
import concourse.mybir as mybir
import concourse.tile as tile
from concourse.bass import ds, ts
from concourse.bass_utils import run_bass_kernel_spmd

BF16 = mybir.dt.bfloat16
F32 = mybir.dt.float32

T = 2048
D = 1024
DG = 512          # head-group width (8 heads x 64)
NH = 8            # heads per core
DH = 64
P = 128
NKO = D // P      # 8 contraction tiles for projections
W = 512           # tq chunk width
NCH = T // W      # 4 chunks
NTC = W // P      # 4 t-tiles per chunk
NPAIR = NH // 2   # 4 head pairs

_CACHED = None  # (nc, input names) -- build/trace once per process


def _build_kernel():
    nc = bacc.Bacc("TRN2", target_bir_lowering=False, debug=False)

    # inputs come host-tiled so every DMA reads contiguous HBM
    xT_d = nc.dram_tensor("xT", [NKO, NCH, P, W], BF16, kind="ExternalInput").ap()
    wqT_d = nc.dram_tensor("wqT", [NKO, P, DG], BF16, kind="ExternalInput").ap()
    wkT_d = nc.dram_tensor("wkT", [NKO, P, DG], BF16, kind="ExternalInput").ap()
    wvT_d = nc.dram_tensor("wvT", [NKO, P, DG], BF16, kind="ExternalInput").ap()
    woT_d = nc.dram_tensor("woT", [DG // P, P, D], BF16, kind="ExternalInput").ap()
    y_d = nc.dram_tensor("y", [T, D], BF16, kind="ExternalOutput").ap()

    with tile.TileContext(nc) as tc:
        with (
            tc.tile_pool(name="const", bufs=1) as const,
            tc.tile_pool(name="sb", bufs=1) as sb,
            tc.tile_pool(name="work", bufs=6) as work,
            tc.tile_pool(name="wnorm", bufs=4) as wnorm,
            tc.tile_pool(name="ysbp", bufs=8) as ysbp,
            tc.tile_pool(name="ps", bufs=2, space="PSUM") as psp,
            tc.tile_pool(name="av", bufs=2, space="PSUM") as avp,
            tc.tile_pool(name="pj", bufs=2, space="PSUM") as pjp,
        ):
            # ---- constants ----
            dmy = const.tile([P, W], BF16, tag="dmy")
            nc.vector.memset(dmy, 0.0)
            mskb = const.tile([P, P], BF16, tag="mskb")

            # ---- persistent SBUF tensors ----
            XT = sb.tile([P, NKO, T], BF16, tag="XT")
            WQT = sb.tile([P, NKO, DG], BF16, tag="WQT")
            WKT = sb.tile([P, NKO, DG], BF16, tag="WKT")
            WVT = sb.tile([P, NKO, DG], BF16, tag="WVT")
            WOT = sb.tile([P, DG // P, D], BF16, tag="WOT")
            QT = sb.tile([P, DG // P, T], BF16, tag="QT")
            KT = sb.tile([P, DG // P, T], BF16, tag="KT")
            VA = sb.tile([P, T // P, NH, DH + 1], BF16, tag="VA")
            # one tile per head pair: o_proj's jt-accumulation then only
            # waits on the pair it actually reads
            OGT = [
                sb.tile([P, T], BF16, name=f"OGT{p}", tag=f"OGT{p}")
                for p in range(NPAIR)
            ]

            # warm the PE's HAM clock gate while the first DMAs land:
            # ~12 dummy matmuls keep it busy past the 3.4us activity
            # window so real work starts at 2.4 GHz, not 1.2
            wps = psp.tile([P, 2 * W], F32, tag="s")
            for _ in range(12):
                nc.tensor.matmul(
                    wps[:, 0:W], lhsT=dmy[:, 0:P], rhs=dmy,
                    start=True, stop=True,
                )

            # ---- input DMAs ----
            # a single sequencer takes ~600ns per dma_start issue, so a
            # one-engine stream of 24+ transfers trickles out over 14us
            # and starves the PE during the ramp (each resulting gap also
            # re-throttles the HAM clock, which then needs ~3.4us at half
            # speed to re-warm). Issue the four early input streams from
            # FOUR sequencers in parallel so everything chunk-0 needs is
            # in flight within ~4us.
            for k in range(NKO):
                nc.sync.dma_start(XT[:, k, 0:W], xT_d[k, 0])
                nc.scalar.dma_start(WQT[:, k, :], wqT_d[k])
                nc.gpsimd.dma_start(WKT[:, k, :], wkT_d[k])
            for k in range(NKO):
                eng = nc.sync if k % 2 == 0 else nc.scalar
                eng.dma_start(WVT[:, k, :], wvT_d[k])
            for k in range(NKO):
                nc.sync.dma_start(XT[:, k, ds(W, W)], xT_d[k, 1])
            for j in range(DG // P):
                nc.scalar.dma_start(WOT[:, j, :], woT_d[j])
            for c in range(2, NCH):
                for k in range(NKO):
                    nc.sync.dma_start(XT[:, k, ds(c * W, W)], xT_d[k, c])

            # gpsimd constants AFTER its DMA issues so wk goes out first;
            # neither is needed until attention starts.
            # multiplicative causal mask for diag tiles: 1 where tq >= tk
            nc.gpsimd.memset(mskb, 1.0)
            nc.gpsimd.affine_select(
                out=mskb, in_=mskb,
                compare_op=mybir.AluOpType.is_ge,
                fill=0.0, base=0,
                pattern=[[1, P]], channel_multiplier=-1,
            )
            # v_aug ones column
            nc.gpsimd.memset(VA[:, :, :, DH], 1.0)

            # ---- projection / o_proj emitters (also used as PE filler) ----
            def proj_qk(wsb, dst, c, dg):
                ps = pjp.tile([P, W], F32, tag="pj")
                for k in range(NKO):
                    nc.tensor.matmul(
                        ps,
                        lhsT=wsb[:, k, ts(dg, P)],
                        rhs=XT[:, k, ds(c * W, W)],
                        start=(k == 0), stop=(k == NKO - 1),
                    )
                nc.vector.tensor_copy(dst[:, dg, ds(c * W, W)], ps)

            def proj_v(c, tl):
                tt = c * NTC + tl
                ps = pjp.tile([P, W], F32, tag="pj")
                for k in range(NKO):
                    nc.tensor.matmul(
                        ps,
                        lhsT=XT[:, k, ts(tt, P)],
                        rhs=WVT[:, k, :],
                        start=(k == 0), stop=(k == NKO - 1),
                    )
                nc.vector.tensor_copy(
                    VA[:, tt, :, 0:DH],
                    ps.rearrange("p (h d) -> p h d", h=NH),
                )

            def proj_groups(c):
                gs = []
                for dg in range(DG // P):
                    gs.append(lambda dg=dg: proj_qk(WQT, QT, c, dg))
                for dg in range(DG // P):
                    gs.append(lambda dg=dg: proj_qk(WKT, KT, c, dg))
                for tl in range(NTC):
                    gs.append(lambda tl=tl: proj_v(c, tl))
                return gs

            def oproj_tt(c, tl):
                tt = c * NTC + tl
                last = c == NCH - 1
                ysb = ysbp.tile([P, D], BF16, tag="ysb")
                for piece in range(2):
                    ps = pjp.tile([P, W], F32, tag="pj")
                    for jt in range(DG // P - 1):
                        nc.tensor.matmul(
                            ps,
                            lhsT=OGT[jt][:, ts(tt, P)],
                            rhs=WOT[:, jt, ds(piece * W, W)],
                            start=(jt == 0), stop=False,
                        )
                    jt = DG // P - 1
                    nc.tensor.matmul(
                        ps,
                        lhsT=OGT[jt][:, ts(tt, P)],
                        rhs=WOT[:, jt, ds(piece * W, W)],
                        start=False, stop=True,
                    )
                    nc.vector.tensor_copy(ysb[:, ds(piece * W, W)], ps)
                    # per-piece issue, split across DMA queues: transfer
                    # time gates both ysb reuse and the kernel's tail.
                    # for the last chunk, alternate issuing engines so the
                    # final drain is not serialized on one sequencer
                    nsplit = 4 if last else 2
                    wq_ = W // nsplit
                    for q in range(nsplit):
                        o = piece * W + q * wq_
                        eng = nc.scalar if last and q % 2 else nc.sync
                        eng.dma_start(
                            y_d[ts(tt, P), ds(o, wq_)], ysb[:, ds(o, wq_)])

            def oproj_groups(c):
                return [lambda tl=tl: oproj_tt(c, tl) for tl in range(NTC)]

            # ---- attention ----
            LAG = 2  # j2-iterations of score/exp lookahead before each AV

            def emit_av_quad(pair, avA, avB, quad, jmax):
                # both heads, both j's of the pair; per-head adjacency
                # keeps same-psum-bank matmuls back-to-back
                for h, av in ((0, avA), (1, avB)):
                    for j, et, off, boff, w in quad:
                        nc.tensor.matmul(
                            av[0:DH + 1, ds(off, w)],
                            lhsT=VA[:, j, 2 * pair + h, :],
                            rhs=et[:, ds(h * boff, w)],
                            start=(j == 0),
                            stop=(j == jmax),
                        )

            # The normalize chain (den -> 1/den -> broadcast -> multiply)
            # is latency-heavy and the DVE queue is strict FIFO: issuing
            # it inline at a pair boundary parks the reciprocal/multiply
            # at the DVE queue head waiting on the pair's last AV matmul,
            # blocking the projection psum-evacuation copies behind it
            # (which stalls the PE on psum banks). Instead the chain is
            # staged: den copies go to ACT at the pair boundary (zero
            # blocking there), and the DVE/GPSIMD stages are deferred via
            # post_q -- one stage per j2-iteration of the NEXT pair -- so
            # every stage's input is already complete when it dequeues.
            post_q = []

            def normalize_start(avA, avB, dstA, dstB):
                state = {}

                def s0():
                    # by now the pair's last AV long completed: these
                    # dequeue with their semaphores already fired
                    denA = wnorm.tile([1, W], F32, tag="den")
                    denB = wnorm.tile([1, W], F32, tag="den")
                    nc.vector.tensor_copy(denA, avA[DH:DH + 1, :])
                    nc.vector.tensor_copy(denB, avB[DH:DH + 1, :])
                    state["den"] = (denA, denB)

                def s1():
                    denA, denB = state["den"]
                    recA = wnorm.tile([1, W], F32, tag="rec")
                    recB = wnorm.tile([1, W], F32, tag="rec")
                    nc.vector.reciprocal_approx_fast(recA, denA)
                    nc.vector.reciprocal_approx_fast(recB, denB)
                    bcbA = wnorm.tile([DH, W], F32, tag="bcb")
                    bcbB = wnorm.tile([DH, W], F32, tag="bcb")
                    nc.gpsimd.partition_broadcast(bcbA, recA)
                    nc.gpsimd.partition_broadcast(bcbB, recB)
                    state["bcb"] = (bcbA, bcbB)

                def s2():
                    bcbA, bcbB = state["bcb"]
                    # multiply straight from psum: the av banks free here
                    nc.vector.tensor_mul(dstA, avA[0:DH, :], bcbA)
                    nc.vector.tensor_mul(dstB, avB[0:DH, :], bcbB)

                post_q.extend([s0, s1, s2])

            def pump_post():
                if post_q:
                    post_q.pop(0)()

            def attention_pair(pair, c, pull_filler):
                jmax = (c + 1) * NTC - 1
                avA = avp.tile([P, W], F32, tag="av")
                avB = avp.tile([P, W], F32, tag="av")
                pend = []
                for j2 in range((jmax + 1) // 2):
                    quad = []
                    for j in (2 * j2, 2 * j2 + 1):
                        off = max(0, j * P - c * W)
                        w = W - off
                        lo = max(c * W, j * P)
                        diag = j * P >= c * W
                        boff = W
                        ps = psp.tile([P, 2 * W], F32, tag="s")
                        for h in range(2):
                            nc.tensor.matmul(
                                ps[:, ds(h * boff, w)],
                                lhsT=KT[h * DH:(h + 1) * DH, pair, ts(j, P)],
                                rhs=QT[h * DH:(h + 1) * DH, pair, ds(lo, w)],
                                start=True, stop=True,
                            )
                        quad.append((j, ps, off, boff, w, diag))
                    done = []
                    for j, ps, off, boff, w, diag in quad:
                        et = work.tile([P, 2 * W], BF16, tag="et")
                        if w <= 256:
                            # narrow diag tiles: two gap-free exps beat
                            # one spanning the dead [w, W) columns
                            for h in range(2):
                                nc.scalar.activation(
                                    et[:, ds(h * W, w)], ps[:, ds(h * W, w)],
                                    mybir.ActivationFunctionType.Exp,
                                    scale=0.125,
                                )
                        else:
                            nc.scalar.activation(
                                et[:, 0:W + w], ps[:, 0:W + w],
                                mybir.ActivationFunctionType.Exp,
                                scale=0.125,
                            )
                        if diag:
                            nc.vector.tensor_mul(et[:, 0:P], et[:, 0:P], mskb)
                            nc.vector.tensor_mul(
                                et[:, ds(boff, P)], et[:, ds(boff, P)], mskb)
                        done.append((j, et, off, boff, w))
                    pend.append(done)
                    # pump BEFORE the AV quad: the av-bank-freeing muls
                    # of the previous pair land just ahead of the AV
                    # writes that reuse those banks
                    pump_post()
                    if len(pend) > LAG:
                        # emit TWO quads (8 AVs back-to-back): AVs all
                        # accumulate into the same two psum banks, so
                        # batching costs nothing and halves the number
                        # of row-configuration transitions the PE pays.
                        # fillers then ride on the OPPOSITE iteration
                        # parity, so a typical iteration has only two
                        # config transitions instead of three.
                        emit_av_quad(pair, avA, avB, pend.pop(0), jmax)
                        emit_av_quad(pair, avA, avB, pend.pop(0), jmax)
                    pull_filler()
                # chunk-0 pairs have only 2 iterations -- finish the
                # previous pair's chain before this pair's AV drain
                while post_q:
                    post_q.pop(0)()
                for quad in pend:
                    emit_av_quad(pair, avA, avB, quad, jmax)
                # engines support a shifted output partition base: head B's
                # normalized output goes straight into partitions 64-127
                normalize_start(
                    avA, avB,
                    OGT[pair][0:DH, ds(c * W, W)],
                    OGT[pair][DH:P, ds(c * W, W)],
                )
                if pair == NPAIR - 1 and c == NCH - 1:
                    # final pair gates the last o_proj: run the chain now
                    while post_q:
                        post_q.pop(0)()

            # ---- main schedule ----
            # only q+k of chunk 0 up front: scores don't need v, and the
            # v-projection otherwise serializes attention start behind the
            # late-arriving wv DMAs; their dense back-to-back runs also
            # keep the HAM clock gate warm through the DMA-bound ramp
            for g in proj_groups(0)[:8]:
                g()

            # Filler assignment balances PE work against the scalar
            # engine's exp load, which grows with the causal window:
            # chunk 3 is exp-bound (~50us of ACT vs ~22us of attention
            # matmuls), so ALL deferred o_projs run there; chunks 0-2
            # take the q/k/v projections.
            for c in range(NCH):
                fillers = []
                if c == 0:
                    # chunk-0 v-projection rides as the first filler; the
                    # AVs that need it start an iteration in
                    fillers += proj_groups(0)[8:]
                if c + 1 < NCH:
                    fillers += proj_groups(c + 1)
                if c == NCH - 1:
                    for cc in range(NCH - 1):
                        fillers += oproj_groups(cc)
                total_slots = NPAIR * ((c + 1) * NTC // 2)
                state = {"slot": 0, "done": 0}

                def pull_filler(pull=True):
                    state["slot"] += 1
                    if not pull:
                        return
                    want = min(
                        len(fillers),
                        len(fillers) * state["slot"] // total_slots,
                    )
                    while state["done"] < want:
                        fillers[state["done"]]()
                        state["done"] += 1

                for pair in range(NPAIR):
                    attention_pair(pair, c, pull_filler)
                while state["done"] < len(fillers):
                    fillers[state["done"]]()
                    state["done"] += 1

            for g in oproj_groups(NCH - 1):
                g()

    nc.compile()
    return nc


def _get_nc():
    global _CACHED
    if _CACHED is None:
        _CACHED = _build_kernel()
    return _CACHED


def _shard_inputs(x, wq, wk, wv, wo):
    bf = ml_dtypes.bfloat16

    def tile_w(wT):  # [D, DG] -> [NKO, P, DG]
        return np.ascontiguousarray(wT.reshape(NKO, P, DG)).astype(bf)

    in_maps = []
    for core in range(8):
        b, g = divmod(core, 2)
        gs = slice(g * DG, (g + 1) * DG)
        xT = x[b].T  # [D, T]
        x4 = xT.reshape(NKO, P, NCH, W).transpose(0, 2, 1, 3)
        in_maps.append({
            "xT": np.ascontiguousarray(x4).astype(bf),
            "wqT": tile_w(wq[gs, :].T),
            "wkT": tile_w(wk[gs, :].T),
            "wvT": tile_w(wv[gs, :].T),
            "woT": np.ascontiguousarray(
                wo[:, gs].T.reshape(DG // P, P, D)).astype(bf),
        })
    return in_maps


def kernel(x, wq, wk, wv, wo, _trace=False, _trace_cores=None):
    x = np.asarray(x, dtype=np.float32)
    wq = np.asarray(wq, dtype=np.float32)
    wk = np.asarray(wk, dtype=np.float32)
    wv = np.asarray(wv, dtype=np.float32)
    wo = np.asarray(wo, dtype=np.float32)

    nc = _get_nc()
    in_maps = _shard_inputs(x, wq, wk, wv, wo)
    res = run_bass_kernel_spmd(
        nc, in_maps, core_ids=list(range(8)),
        trace=_trace,
        **({"trace_cores": _trace_cores} if _trace_cores else {}),
    )
    B = x.shape[0]
    y = np.zeros((B, T, D), dtype=np.float32)
    for core in range(8):
        b = core // 2
        y[b] += res.results[core]["y"].astype(np.float32)
    if _trace:
        return y, res
    return y
